# revision 33
# baseline (speedup 1.0000x reference)
"""CapsNet dynamic-routing kernel for 8 trn2 NeuronCores (pure data parallel).

j-basis formulation: u_hat (1344/sample) is never materialized. Per batch
element, with u[n,j] the squashed primary capsules and W[n,m,j,k] the
routing weights:

  s[m,k]  = sum_{n,j} y[m,n,j] W[n,m,j,k],   y[m,n,j] = c[m,n] u[n,j]
  t[m,n]  = sum_j u[n,j] gq[m,n,j],          gq[m,n,j] = sum_k W[n,m,j,k] s[m,k]
  blog   += t * sh,  sh = 1/(1+|s|^2)        (squash identity v = s*sh)
  out[m]  = sqrt(|s|^2) * sh

The shared-weight contractions run on the PE with batch on the moving
dim: s = WS yT (block-diagonal per m-pair), gq = (W_m W_m^T) yT fused so
gq never waits on s, and |s|^2 via a 0/1 k-reduction matmul plus a tiny
PE transpose.  Per-sample products (y = c*u, pd = u*gq, softmax) run
batch-major on DVE/Pool with free-dim broadcasts; DMA xbar transposes
(14 ns per 16x128 tile) convert layouts (y, gq, u); ScalarE does PSUM
evictions, squares, exps.  bf16 data, fp32 accumulation in PSUM / blog /
norms; reciprocal_approx_fast for all reciprocals.

Emission is a skewed software pipeline with per-iteration s/delta phase
split: at step k the phases stage1(k), it0s(k-1), it0d(k-2), it1s(k-3),
it1d(k-4), it2s(k-5), final(k-5) are emitted, keeping 5-6 tiles in
flight; PSUM sizing (8 banks: z+nsqz 2, psS 2, nsqT 1, pg 3) matches the
skew.  GPSIMD/Pool never touches PSUM (hw restriction).  |s|^2 crosses
to batch-major as bf16 (0.3% worst-case on sh, folded into the 2e-2
budget); the final |v|^2 uses nsq*sh^2 = sh*(1-sh).

DMA transposes are issued at fine granularity (128-col pieces for y,
half-block for gq) so each enters the HWDGE FIFO as soon as its slice
is produced -- coarse transposes cause head-of-line blocking in the
DMA queue and cost ~7% end-to-end.

Measured: CoreSim 229,128 ns/core vs 453 us for the previous u_hat-based
kernel; engine busy: DMA 87%, DVE 82%, ScalarE 81%, Pool 75%, PE 67%
(the exclusive DMA pool is the binding resource).  Hardware rel err
~1.1e-2 vs the fp32 reference (gate 2e-2).
"""

import numpy as np

N_CORES = 8
B_TOTAL = 65536
BP = B_TOTAL // N_CORES          # 8192 samples per core
TILE_F = 512                     # tile width (samples)
N_T512 = BP // TILE_F            # 16
CHUNK = 128
NCH = TILE_F // CHUNK            # 4
N_CAP, D_IN, D_U = 7, 30, 8      # n, input dim, j
M_CAP, D_V = 12, 16              # m, k
NJ = N_CAP * D_U                 # 56
MK = M_CAP * D_V                 # 192
MN = M_CAP * N_CAP               # 84  (m-major: cols (m, n))
MNJ = M_CAP * N_CAP * D_U        # 672 (m, n, j)
YW = M_CAP * 8 * D_U             # 768 = (m, n-padded-to-8, j)
NBLK = 6                         # m-pair blocks

_prog_cache = {}


def _build(num_iterations: int, repeats: int = 1, coarse_y: bool = True,
           coarse_gq: bool = True, skip=(), dilate=None, dn=0, pevac=True):
    skip = set(skip)
    import concourse.bass as bass
    import concourse.bacc as bacc
    import concourse.tile as tile
    from concourse import mybir

    f32 = mybir.dt.float32
    f32r = mybir.dt.float32r
    bf16 = mybir.dt.bfloat16
    AX = mybir.AxisListType
    OP = mybir.AluOpType
    ACT = mybir.ActivationFunctionType

    nit = num_iterations
    nc = bacc.Bacc()
    ew = None  # set after engines exist

    xT = nc.declare_dram_parameter("xT", [210, BP], bf16, isOutput=False)
    w1 = nc.declare_dram_parameter("w1", [210, NJ], bf16, isOutput=False)
    bpc = nc.declare_dram_parameter("bpc", [NJ, 1], f32, isOutput=False)
    bo = nc.declare_dram_parameter("bo", [NJ, NJ], bf16, isOutput=False)
    wqp = nc.declare_dram_parameter("wq", [96, 16], bf16, isOutput=False)
    ws = nc.declare_dram_parameter("ws", [128, NBLK * 96], bf16, isOutput=False)
    wg = nc.declare_dram_parameter("wg", [128, NBLK * 128], bf16, isOutput=False)
    wg1 = nc.declare_dram_parameter("wg1", [NJ, NBLK * 128], bf16, isOutput=False)
    wn = nc.declare_dram_parameter("wn", [96, 2 * 16], bf16, isOutput=False)
    wtp = nc.declare_dram_parameter("wt", [128, NBLK * 96], bf16,
                                    isOutput=False)
    out = nc.declare_dram_parameter("out", [BP, M_CAP], f32, isOutput=True)

    ew = nc.vector if pevac else nc.gpsimd
    with tile.TileContext(nc) as tc:
        from contextlib import ExitStack
        with ExitStack() as _stk:
            nc.allow_low_precision(reason="bf16 big passes; fp32 accum in "
                                          "PSUM/blog/norms").__enter__()
            _p = lambda **kw: _stk.enter_context(tc.tile_pool(**kw))
            singles = _p(name="singles", bufs=1)
            s1p = _p(name="s1p", bufs=3)
            ubuf = _p(name="ubuf", bufs=9)
            sbuf2 = _p(name="sbuf2", bufs=3)
            gbuf = _p(name="gbuf", bufs=2)
            pdb = _p(name="pdb", bufs=4)
            ypool = _p(name="ypool", bufs=3)
            ytp = _p(name="ytp", bufs=5)
            sqp = _p(name="sqp", bufs=3)
            smalls = _p(name="smalls", bufs=4)
            psz = _p(name="psz", bufs=1, space="PSUM")
            pss = _p(name="pss", bufs=1, space="PSUM")
            psn = _p(name="psn", bufs=1, space="PSUM")
            psg = _p(name="psg", bufs=1, space="PSUM")
            pstp = _p(name="pstp", bufs=1, space="PSUM")
            # ---- constants ----
            w1a_s = singles.tile([128, NJ], bf16)
            w1b_s = singles.tile([82, NJ], bf16)
            xa_s = singles.tile([128, BP], bf16)
            xb_s = singles.tile([82, BP], bf16)
            bpc_s = singles.tile([NJ, 1], f32)
            bo_s = singles.tile([NJ, NJ], bf16)
            wq_s = singles.tile([96, 16], bf16)
            ws_s = singles.tile([128, NBLK, 96], bf16)
            wg_s = singles.tile([128, NBLK, 128], bf16)
            wg1_s = singles.tile([NJ, NBLK, 128], bf16)
            wn_s = singles.tile([96, 2, 16], bf16)
            wt_s = singles.tile([128, NBLK, 96], bf16)
            qbuf = singles.tile([CHUNK, N_T512, NCH, M_CAP], f32)
            ones_s = singles.tile([CHUNK, 1], f32)
            nc.gpsimd.memset(ones_s, 1.0)
            dscr = singles.tile([CHUNK, 2, TILE_F], bf16)
            dscrf = singles.tile([CHUNK, NCH * N_CAP], f32)
            dst2 = singles.tile([96, TILE_F], f32, space="PSUM",
                                name="dst2") if dilate == "pe" else None
            if dilate:
                nc.gpsimd.memset(dscr, 0.5)
            obuf = singles.tile([CHUNK, N_T512, NCH, M_CAP], f32)
            nc.sync.dma_start(out=w1a_s, in_=w1[0:128, :])
            nc.sync.dma_start(out=w1b_s, in_=w1[128:210, :])
            nc.sync.dma_start(out=xa_s, in_=xT[0:128, :])
            nc.sync.dma_start(out=xb_s, in_=xT[128:210, :])
            nc.sync.dma_start(out=bpc_s, in_=bpc[:, :])
            nc.sync.dma_start(out=bo_s, in_=bo[:, :])
            nc.sync.dma_start(out=wq_s, in_=wqp[:, :])
            nc.sync.dma_start(out=ws_s.rearrange("p a b -> p (a b)"),
                              in_=ws[:, :])
            nc.sync.dma_start(out=wg_s.rearrange("p a b -> p (a b)"),
                              in_=wg[:, :])
            nc.sync.dma_start(out=wg1_s.rearrange("p a b -> p (a b)"),
                              in_=wg1[:, :])
            nc.sync.dma_start(out=wn_s.rearrange("p a b -> p (a b)"),
                              in_=wn[:, :])
            nc.sync.dma_start(out=wt_s.rearrange("p a b -> p (a b)"),
                              in_=wtp[:, :])


            # ---------- per-tile phase functions (software pipeline) ----
            def ph_stage1(st):
                t = st["t"]
                c0 = t * TILE_F
                z = psz.tile([NJ, TILE_F], f32, tag="z", name="z")
                nc.tensor.matmul(z, w1a_s, xa_s[:, c0:c0 + TILE_F],
                                 start=True, stop=False)
                nc.tensor.matmul(z, w1b_s, xb_s[:, c0:c0 + TILE_F],
                                 start=False, stop=True)
                sq = s1p.tile([NJ, TILE_F], bf16, tag="sq", name="sq")
                nc.scalar.activation(out=sq, in_=z, func=ACT.Square,
                                     bias=bpc_s, scale=1.0)
                nsqz = psz.tile([NJ, TILE_F], f32, tag="nsqz", name="nsqz")
                nc.tensor.matmul(nsqz, bo_s, sq,
                                 start=True, stop=True)
                pf = s1p.tile([NJ, TILE_F], f32, tag="pf", name="pf")
                nc.scalar.add(pf, nsqz, 1.0)
                fz = s1p.tile([NJ, TILE_F], f32, tag="fz", name="fz")
                nc.vector.reciprocal_approx_fast(out=fz, in_=pf)
                uT64 = s1p.tile([64, TILE_F], bf16, tag="uT64", name="uT64")
                uT = uT64[0:NJ, :]
                ew.memset(uT64, 0.0)
                nc.vector.scalar_tensor_tensor(
                    out=uT, in0=z, scalar=bpc_s, in1=fz,
                    op0=OP.add, op1=OP.mult)
                u2a = ubuf.tile([CHUNK, NCH, 64], bf16, tag="u2a", name="u2a")
                nc.sync.dma_start_transpose(out=u2a, in_=uT64)
                u2f = ubuf.tile([CHUNK, TILE_F], bf16, tag="u2f",
                                name="u2f")
                nc.scalar.copy(u2f[0:64, :], uT64)
                nc.scalar.copy(u2f[64:128, :], uT64)
                st["uT"] = uT
                st["u2"] = u2a
                st["u2f"] = u2f
                for i in range(dn):
                    src, dd = dscr[:, 0, :], dscr[:, 1, :]
                    if dilate == "dve":
                        nc.vector.tensor_copy(out=dd, in_=src)
                    elif dilate == "pool":
                        nc.gpsimd.tensor_copy(out=dd, in_=src)
                    elif dilate == "sc":
                        nc.scalar.copy(dd, src)
                    elif dilate == "pe":
                        nc.tensor.matmul(dst2, wt_s[:, 0, :], src,
                                         start=True, stop=True)
                    elif dilate == "dma":
                        nc.sync.dma_start_transpose(
                            out=dd.rearrange("p (c q) -> p c q", c=NCH),
                            in_=src)
                    elif dilate == "zs":
                        nc.vector.tensor_reduce(
                            dscrf,
                            src[:, 0:NCH * MN].rearrange(
                                "p (c m n) -> p c n m", c=NCH, m=M_CAP),
                            axis=AX.X, op=OP.add)
                    elif dilate == "ydve":
                        nc.vector.tensor_tensor(
                            out=dd[:, 0:NCH * MN].rearrange(
                                "p (c m n) -> p c m n", c=NCH, m=M_CAP),
                            in0=src[:, 0:NCH * MN].rearrange(
                                "p (c m n) -> p c m n", c=NCH, m=M_CAP),
                            in1=src[:, 0:NCH * N_CAP].rearrange(
                                "p (c n) -> p c n", c=NCH).unsqueeze(2)
                                .broadcast_to([CHUNK, NCH, M_CAP, N_CAP]),
                            op=OP.mult)
                    elif dilate == "ctpool":
                        nc.gpsimd.tensor_tensor(
                            out=dd[:, 0:NCH * MN].rearrange(
                                "p (c m n) -> p c m n", c=NCH, m=M_CAP),
                            in0=src[:, 0:NCH * MN].rearrange(
                                "p (c m n) -> p c m n", c=NCH, m=M_CAP),
                            in1=src[:, 0:NCH * N_CAP].rearrange(
                                "p (c n) -> p c n", c=NCH).unsqueeze(2)
                                .broadcast_to([CHUNK, NCH, M_CAP, N_CAP]),
                            op=OP.mult)

            def nsq_tail(nsqB):
                p1 = smalls.tile([CHUNK, NCH, 12], f32, tag="p1", name="p1")
                nc.scalar.add(p1, nsqB[:, :, 0:12], 1.0)
                sh = smalls.tile([CHUNK, NCH * 12], f32, tag="sh",
                                 name="sh", bufs=8)
                nc.vector.reciprocal_approx_fast(
                    out=sh, in_=p1.rearrange("p c m -> p (c m)"))
                return sh

            def nsq_of(ps):
                # last iter only: |s|^2 from s via squares + 0/1 k-reduce
                sqS = sqp.tile([96, 2, TILE_F], bf16, tag="sqS",
                                  name="sqS")
                nc.scalar.activation(out=sqS[:, 0, :], in_=ps[0],
                                     func=ACT.Square)
                nc.scalar.activation(out=sqS[:, 1, :], in_=ps[1],
                                     func=ACT.Square)
                nsqT = psn.tile([16, TILE_F], f32, tag="nsqT", name="nsqT")
                for h in range(2):
                    nc.tensor.matmul(nsqT, wn_s[:, h, :], sqS[:, h, :],
                                     start=(h == 0), stop=(h == 1))
                nsqS = smalls.tile([16, TILE_F], bf16, tag="nsqS",
                                   name="nsqS")
                nc.scalar.copy(nsqS, nsqT)
                nsqB = smalls.tile([CHUNK, NCH, 16], bf16, tag="nsqB",
                                   name="nsqB")
                nc.sync.dma_start_transpose(out=nsqB, in_=nsqS)
                return nsq_tail(nsqB)

            def tblock(st, src_ap, it, with_nsq=False):
                # gq = G y in PSUM; pd = u (.) gq on DVE straight from PSUM;
                # t[m,n] = sum_j pd via 0/1 matmuls into one [96, F]
                # accumulator (row m*8+n); only t crosses to batch-major.
                u2f = st["u2f"]
                pst = pstp.tile([96, TILE_F], f32, tag="pst", name="pst")
                pdfs = []
                for p in range(NBLK // 2):
                    pg = psg.tile([CHUNK, 2, TILE_F], f32, tag="pg",
                                  name="pg")
                    for b2 in range(2):
                        b = 2 * p + b2
                        if it == 0:
                            nc.tensor.matmul(pg[:, b2, :], wg1_s[:, b, :],
                                             src_ap, start=True, stop=True)
                        else:
                            nc.tensor.matmul(
                                pg[:, b2, :], wg_s[:, b, :],
                                src_ap.rearrange("p (c b) q -> p c b q",
                                                 c=NCH)[:, :, b, :],
                                start=True, stop=True)
                    pdf = pdb.tile([CHUNK, 2, TILE_F], bf16, tag="pdf",
                                   name="pdf")
                    for b2 in range(2):
                        nc.vector.tensor_tensor(out=pdf[:, b2, :],
                                                in0=pg[:, b2, :], in1=u2f,
                                                op=OP.mult)
                    pdfs.append(pdf)
                for b in range(NBLK):
                    nc.tensor.matmul(pst, wt_s[:, b, :],
                                     pdfs[b // 2][:, b % 2, :],
                                     start=(b == 0), stop=(b == NBLK - 1))
                if with_nsq:
                    tS = sqp.tile([112, TILE_F], bf16, tag="tSn",
                                  name="tS")
                    nc.scalar.copy(tS[0:96, :], pst)
                    nsqT = psn.tile([16, TILE_F], f32, tag="nsqT",
                                    name="nsqT")
                    nc.tensor.matmul(nsqT, wq_s, tS[0:96, :], start=True,
                                     stop=True)
                    nc.scalar.copy(tS[96:112, :], nsqT)
                    tB = gbuf.tile([CHUNK, NCH, 112], bf16, tag="tBn",
                                   name="tB", bufs=6)
                    nc.sync.dma_start_transpose(out=tB, in_=tS)
                    return tB
                tS = sqp.tile([96, TILE_F], bf16, tag="tS", name="tS")
                nc.scalar.copy(tS, pst)
                tB = gbuf.tile([CHUNK, NCH, 96], bf16, tag="tB", name="tB",
                               bufs=6)
                nc.sync.dma_start_transpose(out=tB, in_=tS)
                return tB

            def dst_of(tB, sh, it):
                tv = tB[:, :, 0:96].rearrange("p c (m n1) -> p c m n1",
                                              m=M_CAP)[:, :, :, 0:N_CAP]
                shv = sh.rearrange("p (c m) -> p c m", c=NCH)
                dst = smalls.tile([CHUNK, NCH * MN], f32,
                                  tag="blog" if it == 0 else "d_t",
                                  name="dst", bufs=8 if it == 0 else 3)
                dv = dst.rearrange("p (c m n) -> p c m n", c=NCH, m=M_CAP)
                ew.tensor_tensor(
                    out=dv, in0=tv,
                    in1=shv.unsqueeze(3)
                        .broadcast_to([CHUNK, NCH, M_CAP, N_CAP]),
                    op=OP.mult)
                return dst

            def ph_it0g(st):
                # it0 delta front; |s|^2 = sum_n t[m,n] / 12 (c uniform).
                # t and nsq share one [112, F] tile and one transpose.
                tc2 = tblock(st, st["uT"], 0, with_nsq=True)
                st["tB"] = tc2
                st["nsqB"] = tc2

            def ph_it0t(st):
                tc2 = st["tB"]
                p1 = smalls.tile([CHUNK, NCH, 12], f32, tag="p1", name="p1")
                nc.scalar.add(p1, tc2[:, :, 96:108], 1.0)
                sh = smalls.tile([CHUNK, NCH * 12], f32, tag="sh",
                                 name="sh", bufs=8)
                nc.vector.reciprocal_approx_fast(
                    out=sh, in_=p1.rearrange("p c m -> p (c m)"))
                st["sh"] = sh
                if nit > 1:
                    st["blog"] = dst_of(tc2, sh, 0)

            def ph_itka(st, it):
                t = st["t"]
                blog = st["blog"]
                e = smalls.tile([CHUNK, NCH * MN], bf16, tag="e", name="e")
                nc.scalar.activation(out=e, in_=blog, func=ACT.Exp)
                zs = smalls.tile([CHUNK, NCH * N_CAP], f32, tag="zs",
                                 name="zs")
                nc.vector.tensor_reduce(
                    zs, e.rearrange("p (c m n) -> p c n m", c=NCH, m=M_CAP),
                    axis=AX.X, op=OP.add)
                rz = smalls.tile([CHUNK, NCH * N_CAP], f32, tag="rz",
                                 name="rz")
                nc.vector.reciprocal_approx_fast(out=rz, in_=zs)
                c_t = smalls.tile([CHUNK, NCH * MN], bf16, tag="c_t",
                                  name="c_t", bufs=6)
                ew.tensor_tensor(
                    out=c_t.rearrange("p (c m n) -> p c m n", c=NCH,
                                      m=M_CAP),
                    in0=e.rearrange("p (c m n) -> p c m n", c=NCH, m=M_CAP),
                    in1=rz.rearrange("p (c n) -> p c n", c=NCH)
                        .unsqueeze(2)
                        .broadcast_to([CHUNK, NCH, M_CAP, N_CAP]),
                    op=OP.mult)
                y = ypool.tile([CHUNK, NCH, YW], bf16, tag="y", name="y")
                cv = c_t.rearrange("p (c m n) -> p c m n", c=NCH, m=M_CAP)
                yv = y.rearrange("p c (m n j) -> p c m n j", m=M_CAP, n=8)
                ew.memset(
                    y.rearrange("p c (g n j) -> p (c g) n j", n=8, j=D_U)
                     [:, :, N_CAP:8, :], 0.0)
                u2 = st["u2"]
                MH = M_CAP // 2
                for half, eng in ((0, nc.vector), (1, nc.gpsimd)):
                    eng.tensor_tensor(
                        out=yv[:, :, half * MH:(half + 1) * MH, 0:N_CAP, :],
                        in0=cv[:, :, half * MH:(half + 1) * MH].unsqueeze(4)
                            .broadcast_to([CHUNK, NCH, MH, N_CAP, D_U]),
                        in1=u2[:, :, 0:NJ]
                            .rearrange("p c (n j) -> p c n j", n=N_CAP)
                            .unsqueeze(2)
                            .broadcast_to([CHUNK, NCH, MH, N_CAP, D_U]),
                        op=OP.mult)
                yT = ytp.tile([CHUNK, NCH * NBLK, CHUNK], bf16, tag="yT",
                              name="yT")
                yf = y.rearrange("p c w -> p (c w)")
                nc.sync.dma_start_transpose(out=yT, in_=yf)
                st["yT"] = yT
                st["c_t"] = c_t

            def ph_itkg(st, it):
                st["tB"] = tblock(st, st["yT"], it)

            def ph_itkt(st, it):
                # |s|^2 = sum_n c[m,n] t[m,n]  (batch-major DVE), then sh,
                # blog += t*sh
                c_t, tB = st["c_t"], st["tB"]
                tv = tB.rearrange("p c (m n1) -> p c m n1",
                                  m=M_CAP)[:, :, :, 0:N_CAP]
                cv = c_t.rearrange("p (c m n) -> p c m n", c=NCH, m=M_CAP)
                ct = smalls.tile([CHUNK, NCH * MN], bf16, tag="ctp",
                                 name="ct")
                nc.gpsimd.tensor_tensor(
                    out=ct.rearrange("p (c m n) -> p c m n", c=NCH,
                                     m=M_CAP),
                    in0=tv, in1=cv, op=OP.mult)
                nsqb = smalls.tile([CHUNK, NCH, 12], f32, tag="nsqq",
                                   name="nsqb")
                nc.vector.tensor_reduce(
                    nsqb, ct.rearrange("p (c m n) -> p c m n", c=NCH,
                                       m=M_CAP),
                    axis=AX.X, op=OP.add)
                p1 = smalls.tile([CHUNK, NCH, 12], f32, tag="p1", name="p1")
                nc.scalar.add(p1, nsqb, 1.0)
                sh = smalls.tile([CHUNK, NCH * 12], f32, tag="sh",
                                 name="sh", bufs=8)
                nc.vector.reciprocal_approx_fast(
                    out=sh, in_=p1.rearrange("p c m -> p (c m)"))
                d_t = dst_of(tB, sh, it)
                nblog = smalls.tile([CHUNK, NCH * MN], f32, tag="blog",
                                    name="nblog", bufs=8)
                nc.gpsimd.tensor_add(nblog, st["blog"], d_t)
                st["blog"] = nblog

            def ph_itkb(st, it):
                # last iter: s-matmuls + |s|^2 for the final output norm
                yT = st["yT"]
                ps = [pss.tile([96, TILE_F], f32, name=f"psS{h}",
                               tag=f"psS{h}") for h in range(2)]
                for h in range(2):
                    for i, b in enumerate((3 * h, 3 * h + 1, 3 * h + 2)):
                        nc.tensor.matmul(
                            ps[h], ws_s[:, b, :],
                            yT.rearrange("p (c b) q -> p c b q", c=NCH)
                              [:, :, b, :],
                            start=(i == 0), stop=(i == 2))
                st["sh"] = nsq_of(ps)

            def ph_final(st):
                # q = nsq*sh^2 = sh*(1-sh)  since nsq*sh = 1-sh
                t, sh = st["t"], st["sh"]
                a_t = smalls.tile([CHUNK, NCH * 12], f32, tag="a_t",
                                  name="a_t")
                nc.scalar.activation(out=a_t, in_=sh, func=ACT.Square)
                nc.vector.tensor_sub(
                    qbuf[:, t, :, :].rearrange("p c m -> p (c m)"),
                    sh, a_t)

            # ---------- skewed software pipeline over tiles --------------
            # stage1(k), it0g(k-1), it0t(k-2), then per iter j>=1:
            # a(k-3j) softmax+y+transpose, g(k-3j-1) gq/t, t(k-3j-2)
            # sh+blog update; the last iter runs b(k-3j-1) = s-matmuls+nsq
            # instead of g/t.  Every cross-engine hop gets >= 1 full step
            # of unrelated work as latency slack.
            order = [t % N_T512 for t in range(N_T512 * repeats)]
            N = len(order)
            states = {}
            fin_off = 2 if nit == 1 else 3 * (nit - 1) + 2
            for k in range(N + fin_off + 1):
                if k < N:
                    states[k] = {"t": order[k], "k": k}
                    ph_stage1(states[k])
                if 0 <= k - 1 < N:
                    ph_it0g(states[k - 1])
                if 0 <= k - 2 < N:
                    ph_it0t(states[k - 2])
                for it in range(1, nit):
                    i = k - 3 * it
                    if 0 <= i < N:
                        ph_itka(states[i], it)
                    if it < nit - 1:
                        i = k - 3 * it - 1
                        if 0 <= i < N:
                            ph_itkg(states[i], it)
                        i = k - 3 * it - 2
                        if 0 <= i < N:
                            ph_itkt(states[i], it)
                    else:
                        i = k - 3 * it - 1
                        if 0 <= i < N:
                            ph_itkb(states[i], it)
                i = k - fin_off
                if 0 <= i < N:
                    ph_final(states[i])
                    del states[i]

            nc.scalar.activation(out=obuf, in_=qbuf, func=ACT.Sqrt)
            nc.sync.dma_start(
                out=out.rearrange("(g p) m -> p g m", p=CHUNK,
                                  g=BP // CHUNK),
                in_=obuf.rearrange("p a c m -> p (a c) m"))
    nc.compile()
    return nc


def _prep_weights(W_pc, b_pc, W):
    import ml_dtypes
    W1 = np.zeros((210, NJ), np.float32)
    BO = np.zeros((NJ, NJ), np.float32)
    for n in range(N_CAP):
        W1[n * D_IN:(n + 1) * D_IN, n * D_U:(n + 1) * D_U] = W_pc[n].T
        BO[n * D_U:(n + 1) * D_U, n * D_U:(n + 1) * D_U] = 1.0
    BPC = b_pc.reshape(NJ, 1).astype(np.float32)

    WS = np.zeros((128, NBLK, 96), np.float32)
    WG = np.zeros((128, NBLK, 128), np.float32)
    WG1 = np.zeros((NJ, NBLK, 128), np.float32)
    for n in range(N_CAP):
        for j in range(D_U):
            for m in range(M_CAP):
                h, mh = m // 6, m % 6
                for k in range(D_V):
                    w = W[n, m, j, k]
                    b, m2 = m // 2, m % 2
                    WS[m2 * 64 + n * D_U + j, b, mh * 16 + k] = w
    WN = np.zeros((96, 2, 16), np.float32)
    for h in range(2):
        for mh in range(6):
            for k in range(D_V):
                WN[mh * 16 + k, h, 6 * h + mh] = 1.0
    # gq = (W_m W_m^T) y composed per output capsule m:
    # G_m[(n',j'), (n,j)] = sum_k W[n',m,j',k] W[n,m,j,k]
    for b in range(NBLK):
        for m2 in range(2):
            m = 2 * b + m2
            G = np.einsum('abk,njk->abnj', W[:, m],
                          W[:, m]).reshape(NJ, NJ)
            WG[m2 * 64:m2 * 64 + NJ, b, m2 * 64:m2 * 64 + NJ] = G
            WG1[:, b, m2 * 64:m2 * 64 + NJ] = G / M_CAP
    # t-reduce: block b maps row (m2*64 + n*8 + j) -> col 16*b + 8*m2 + n
    # (one [96, F] accumulation group; cols 16b+7, 16b+15 stay zero)
    WT = np.zeros((128, NBLK, 96), np.float32)
    for b in range(NBLK):
        for m2 in range(2):
            for n in range(N_CAP):
                for j in range(D_U):
                    WT[m2 * 64 + n * D_U + j, b, 16 * b + 8 * m2 + n] = 1.0
    # it0 norm: |s|^2 = sum_n t[m,n] / 12;  row m*8+n -> col m
    WQ = np.zeros((96, 16), np.float32)
    for m in range(M_CAP):
        for n in range(N_CAP):
            WQ[m * 8 + n, m] = 1.0 / M_CAP
    tobf = lambda a: a.astype(ml_dtypes.bfloat16)
    return (tobf(W1), BPC, tobf(BO), tobf(WQ),
            tobf(WS.reshape(128, NBLK * 96)),
            tobf(WG.reshape(128, NBLK * 128)),
            tobf(WG1.reshape(NJ, NBLK * 128)),
            tobf(WN.reshape(96, 32)), tobf(WT.reshape(128, NBLK * 96)))


def _make_in_maps(x, W_pc, b_pc, W):
    W1, BPC, BO, WQ, WS, WG, WG1, WN, WT = _prep_weights(W_pc, b_pc, W)
    import ml_dtypes
    xt = np.ascontiguousarray(x.T).astype(ml_dtypes.bfloat16)  # [210, B]
    in_maps = []
    for i in range(N_CORES):
        in_maps.append({
            "xT": np.ascontiguousarray(xt[:, i * BP:(i + 1) * BP]),
            "w1": W1, "bpc": BPC, "bo": BO,
            "wq": WQ, "ws": WS, "wg": WG, "wg1": WG1,
            "wn": WN, "wt": WT,
        })
    return in_maps


def kernel(x, W_pc, b_pc, W, num_iterations, _trace=False):
    from concourse.bass_utils import run_bass_kernel_spmd

    x = np.asarray(x, np.float32)
    W_pc = np.asarray(W_pc, np.float32)
    b_pc = np.asarray(b_pc, np.float32)
    W = np.asarray(W, np.float32)
    nit = int(num_iterations)
    assert x.shape == (B_TOTAL, 210)

    key = nit
    if key not in _prog_cache:
        _prog_cache[key] = _build(nit)
    nc = _prog_cache[key]

    in_maps = _make_in_maps(x, W_pc, b_pc, W)
    res = run_bass_kernel_spmd(nc, in_maps, list(range(N_CORES)),
                               trace=_trace)
    outs = [res.results[i]["out"] for i in range(N_CORES)]
    full = np.concatenate(outs, axis=0)
    if _trace:
        kernel._last_exec_time_ns = res.exec_time_ns
        kernel._last_results = res
    return full

